# revision 18
# baseline (speedup 1.0000x reference)
import sys
import os

sys.path.insert(0, "/opt/trn_rl_repo")

import numpy as np
import ml_dtypes

import concourse.bacc as bacc
import concourse.bass as bass
import concourse.tile as tile
from concourse import mybir
from concourse.bass_utils import run_bass_kernel_spmd

f32 = mybir.dt.float32
bf16 = mybir.dt.bfloat16
u32 = mybir.dt.uint32
i32 = mybir.dt.int32

# problem geometry (hardcoded; kernel.py must be self-contained)
H = W = 64
C = 3
K = 32
PAD = 10
OH = H + 2 * PAD - K + 1          # 53
L = OH * OH                       # 2809
MT = 22                           # m-tiles of 128 rows: 22*128 = 2816 >= L
LP = MT * 128
D = C * K * K                     # 3072
DSTEP = D // 128                  # 24
N_MEM = 20000
NCORES = 8
NK = N_MEM // NCORES              # 2500 keys per core
NSUB = 5                          # key column tiles of 512
SUBW = 512
NKP = NSUB * SUBW                 # 2560 (padded)
VCH = 3                           # patch columns folded per chunk

# ---- packed input blob layout (bf16 element offsets) ----
# [vals | eyes | qh | ql | kh | kl | f32 tail (bitcast): bias | bases | ones]
SZ_VALS = (NK + 1) * D            # 7,683,072  (vals first: indirect src @ 0)
SZ_EYES = OH * K * 84
SZ_Q1 = 128 * DSTEP * 128         # one m-tile of queries
SZ_Q = MT * SZ_Q1
SZ_K1 = 128 * DSTEP * SUBW        # one si-tile of keys
SZ_K = NSUB * SZ_K1
OV = 0
OE = OV + SZ_VALS
OQH = OE + SZ_EYES
OQL = OQH + SZ_Q
OKH = OQL + SZ_Q
OKL = OKH + SZ_K
OF = OKL + SZ_K                   # f32 tail starts here (even offset)
# f32-tail offsets, in f32 elements relative to OF
OB = 0
OBA = OB + 128 * NKP
OO = OBA + 128 * NSUB
NF = OO + 64
NB = OF + 2 * NF

GROUPS = [[0, 1], [2, 3], [4]]

TRACE = False
DEBUG_GI = False                  # adds a 'gi' debug output (costly on wall)
LAST_EXEC_NS = None
LAST_RESULTS = None

_NC_CACHE = {}


def _build(collectives=True, debug_gi=False):
    nc = bacc.Bacc("TRN2", target_bir_lowering=False, debug=False,
                   num_devices=NCORES)

    def allreduce(op, src, dst):
        if collectives:
            nc.gpsimd.collective_compute(
                "AllReduce", op,
                replica_groups=[list(range(NCORES))],
                ins=[src.opt()], outs=[dst.opt()])
        else:
            nc.sync.dma_start(dst[:], src[:])

    bb_d = nc.dram_tensor("bb", [NB], bf16, kind="ExternalInput")
    out_d = nc.dram_tensor("out", [64, C, 64], f32, kind="ExternalOutput")
    if debug_gi:
        gi_d = nc.dram_tensor("gi", [128, MT], f32, kind="ExternalOutput")

    def bview(ofs, sz, pat, **kw):
        return bb_d[ofs:ofs + sz].rearrange(pat, **kw)

    def fview(ofs, sz, pat, **kw):
        return bb_d[OF + 2 * ofs:OF + 2 * (ofs + sz)].bitcast(f32).rearrange(
            pat, **kw)

    with tile.TileContext(nc) as tc:
        with (
            tc.tile_pool(name="keys", bufs=1) as kpool,
            tc.tile_pool(name="qp", bufs=2) as qpool,
            tc.tile_pool(name="work", bufs=1) as wpool,
            tc.tile_pool(name="sm", bufs=2) as mpool,
            tc.tile_pool(name="vt", bufs=2) as vpool,
            tc.tile_pool(name="psum", bufs=2, space=bass.MemorySpace.PSUM) as ppool,
            tc.tile_pool(name="dram", bufs=1, space="DRAM") as dpool,
        ):
            tbias = wpool.tile([128, NKP], f32)
            nc.sync.dma_start(
                tbias[:], fview(OB, 128 * NKP, "(p c) -> p c", p=128, c=NKP))
            tbases = wpool.tile([128, NSUB], f32)
            nc.sync.dma_start(
                tbases[:], fview(OBA, 128 * NSUB, "(p c) -> p c",
                                 p=128, c=NSUB))
            tones = wpool.tile([1, 64], f32)
            nc.sync.dma_start(
                tones[:], fview(OO, 64, "(p c) -> p c", p=1, c=64))
            eyes = wpool.tile([OH, K, 84], bf16)
            nc.sync.dma_start(
                eyes[:], bview(OE, SZ_EYES, "(a k y) -> a k y",
                               a=OH, k=K, y=84))

            best = wpool.tile([128, MT], f32)
            bix = wpool.tile([128, MT], f32)

            # ---------------- scan: scores + per-core argmax ----------------
            _scopes = [nc.named_scope("scan")]
            _scopes[-1].__enter__()
            for gidx, group in enumerate(GROUPS):
                g0, glen = group[0], len(group)
                kht = kpool.tile([128, glen, DSTEP, SUBW], bf16)
                klt = kpool.tile([128, glen, DSTEP, SUBW], bf16)
                nc.sync.dma_start(
                    kht[:],
                    bview(OKH + g0 * SZ_K1, glen * SZ_K1, "(s p d w) -> s p d w",
                          s=glen, p=128, d=DSTEP, w=SUBW).transpose([1, 0, 2, 3]))
                nc.sync.dma_start(
                    klt[:],
                    bview(OKL + g0 * SZ_K1, glen * SZ_K1, "(s p d w) -> s p d w",
                          s=glen, p=128, d=DSTEP, w=SUBW).transpose([1, 0, 2, 3]))

                for m in range(MT):
                    qht = qpool.tile([128, DSTEP, 128], bf16)
                    qlt = qpool.tile([128, DSTEP, 128], bf16)
                    nc.sync.dma_start(
                        qht[:], bview(OQH + m * SZ_Q1, SZ_Q1, "(p d w) -> p d w",
                                      p=128, d=DSTEP, w=128))
                    nc.sync.dma_start(
                        qlt[:], bview(OQL + m * SZ_Q1, SZ_Q1, "(p d w) -> p d w",
                                      p=128, d=DSTEP, w=128))

                    for si in range(glen):
                        s = g0 + si
                        acc = ppool.tile([128, SUBW], f32)
                        passes = [(qht, kht), (qht, klt), (qlt, kht)]
                        nmm = DSTEP * len(passes)
                        i = 0
                        for d in range(DSTEP):
                            for (lt, rt) in passes:
                                nc.tensor.matmul(acc[:], lt[:, d, :],
                                                 rt[:, si, d, :],
                                                 start=(i == 0),
                                                 stop=(i == nmm - 1))
                                i += 1

                        sc = mpool.tile([128, SUBW], f32)
                        nc.vector.scalar_tensor_tensor(
                            sc[:], acc[:], 1.0,
                            tbias[:, s * SUBW:(s + 1) * SUBW],
                            op0=mybir.AluOpType.mult,
                            op1=mybir.AluOpType.add)
                        mxv = mpool.tile([128, 8], f32)
                        mxi = mpool.tile([128, 8], u32)
                        nc.vector.max_with_indices(mxv[:], mxi[:], sc[:])
                        nixf = mpool.tile([128, 1], f32)
                        nc.vector.tensor_copy(nixf[:], mxi[:, 0:1])
                        nix2 = mpool.tile([128, 1], f32)
                        nc.vector.tensor_scalar_add(nix2[:], nixf[:],
                                                    tbases[:, s:s + 1])
                        if gidx == 0 and si == 0:
                            nc.vector.tensor_copy(best[:, m:m + 1],
                                                  mxv[:, 0:1])
                            nc.vector.tensor_copy(bix[:, m:m + 1], nix2[:])
                        else:
                            gt = mpool.tile([128, 1], u32)
                            nc.vector.scalar_tensor_tensor(
                                gt[:], mxv[:, 0:1], 1.0, best[:, m:m + 1],
                                op0=mybir.AluOpType.mult,
                                op1=mybir.AluOpType.is_gt)
                            nc.vector.copy_predicated(best[:, m:m + 1],
                                                      gt[:], mxv[:, 0:1])
                            nc.vector.copy_predicated(bix[:, m:m + 1],
                                                      gt[:], nix2[:])

            # ------------- global argmin via AllReduce(max)+(min) -----------
            _scopes[-1].__exit__(None, None, None)
            _scopes.append(nc.named_scope("argminred"))
            _scopes[-1].__enter__()
            cc1 = dpool.tile([128, MT], f32)
            cc2 = dpool.tile([128, MT], f32)
            nc.gpsimd.dma_start(cc1[:], best[:])
            allreduce(mybir.AluOpType.max, cc1, cc2)
            gbest = wpool.tile([128, MT], f32)
            nc.gpsimd.dma_start(gbest[:], cc2[:])

            ge = wpool.tile([128, MT], f32)
            nc.vector.scalar_tensor_tensor(
                ge[:], best[:], 1.0, gbest[:],
                op0=mybir.AluOpType.mult, op1=mybir.AluOpType.is_ge)
            t1 = wpool.tile([128, MT], f32)
            nc.vector.tensor_scalar_add(t1[:], bix[:], -1.0e6)
            t2 = wpool.tile([128, MT], f32)
            nc.vector.scalar_tensor_tensor(
                t2[:], ge[:], 1.0, t1[:],
                op0=mybir.AluOpType.mult, op1=mybir.AluOpType.mult)
            cand = wpool.tile([128, MT], f32)
            nc.vector.tensor_scalar_add(cand[:], t2[:], 1.0e6)

            cc3 = dpool.tile([128, MT], f32)
            cc4 = dpool.tile([128, MT], f32)
            nc.gpsimd.dma_start(cc3[:], cand[:])
            allreduce(mybir.AluOpType.min, cc3, cc4)
            gif = wpool.tile([128, MT], f32)
            nc.gpsimd.dma_start(gif[:], cc4[:])
            if debug_gi:
                nc.sync.dma_start(gi_d[:], gif[:])

            # local row index: owned -> gi - c*2500, else zero row NK;
            # scaled by D to give a flat bf16-blob element offset.
            li = wpool.tile([128, MT], f32)
            nc.vector.tensor_scalar(li[:], gif[:], tbases[:, 0:1], None,
                                    op0=mybir.AluOpType.subtract)
            o1 = wpool.tile([128, MT], f32)
            nc.vector.tensor_scalar(o1[:], li[:], 0.0, None,
                                    op0=mybir.AluOpType.is_ge)
            o2 = wpool.tile([128, MT], f32)
            nc.vector.tensor_scalar(o2[:], li[:], float(NK), None,
                                    op0=mybir.AluOpType.is_lt)
            own = wpool.tile([128, MT], f32)
            nc.vector.scalar_tensor_tensor(
                own[:], o1[:], 1.0, o2[:],
                op0=mybir.AluOpType.mult, op1=mybir.AluOpType.mult)
            d1 = wpool.tile([128, MT], f32)
            nc.vector.tensor_scalar_add(d1[:], li[:], -float(NK))
            d2t = wpool.tile([128, MT], f32)
            nc.vector.scalar_tensor_tensor(
                d2t[:], own[:], 1.0, d1[:],
                op0=mybir.AluOpType.mult, op1=mybir.AluOpType.mult)
            lc = wpool.tile([128, MT], f32)
            nc.vector.tensor_scalar_add(lc[:], d2t[:], float(NK))
            lci = wpool.tile([128, MT], i32)
            nc.vector.tensor_copy(lci[:], lc[:])

            # relayout [128, MT] -> [1, LP] (patch-id order) -> [oh, ow] grid
            gidr = dpool.tile([128, MT], i32)
            nc.sync.dma_start(gidr[:], lci[:])
            gi32 = wpool.tile([1, LP], i32)
            nc.sync.dma_start(gi32[:], gidr.transpose([1, 0])[:])
            dgrid = dpool.tile([OH, OH], i32)
            nc.sync.dma_start(
                dgrid[:], gi32[0:1, 0:L].rearrange("p (a b) -> p a b",
                                                   a=OH, b=OH))
            idxT = wpool.tile([OH, OH], i32)
            nc.sync.dma_start(idxT[:], dgrid[:])

            # --------------------- gather + fold ---------------------------
            _scopes[-1].__exit__(None, None, None)
            _scopes.append(nc.named_scope("gatherfold"))
            _scopes[-1].__enter__()

            vtab = bb_d[0:SZ_VALS].rearrange("(r d) -> r d", r=NK + 1, d=D)
            Wt = wpool.tile([84, OH, C, K], bf16)
            c0 = 0
            while c0 < OH:
                clen = min(VCH, OH - c0)
                vtc = vpool.tile([128, VCH, D], bf16)
                for j in range(clen):
                    nc.gpsimd.indirect_dma_start(
                        out=vtc[0:OH, j, :],
                        out_offset=None,
                        in_=vtab,
                        in_offset=bass.IndirectOffsetOnAxis(
                            ap=idxT[0:OH, c0 + j:c0 + j + 1], axis=0),
                    )
                vtR = vtc[:].rearrange("q g (c ky kx) -> q g c ky kx",
                                       c=C, ky=K, kx=K)
                zp = ppool.tile([84, VCH * C * K], f32)
                for ky in range(K):
                    nc.tensor.matmul(zp[:, 0:clen * C * K],
                                     eyes[0:53, ky, :],
                                     vtR[0:53, 0:clen, :, ky, :],
                                     start=(ky == 0), stop=(ky == K - 1))
                nc.vector.tensor_copy(
                    Wt[0:84, c0:c0 + clen, :, :],
                    zp[:, 0:clen * C * K].rearrange(
                        "p (g c k) -> p g c k", g=clen, c=C, k=K))
                c0 += clen

            # stage B: fold along ow via strided in-place adds
            cl = wpool.tile([84, C, 84], f32)
            nc.vector.memset(cl[:], 0.0)
            for kx in range(K):
                nc.vector.scalar_tensor_tensor(
                    cl[0:84, :, kx:kx + OH],
                    Wt[0:84, :, :, kx].transpose([0, 2, 1]), 1.0,
                    cl[0:84, :, kx:kx + OH],
                    op0=mybir.AluOpType.mult, op1=mybir.AluOpType.add)

            cc5 = dpool.tile([H + 2 * PAD, C, H + 2 * PAD], f32)
            cc6 = dpool.tile([H + 2 * PAD, C, H + 2 * PAD], f32)
            nc.sync.dma_start(cc5[:], cl[:])

            # sum partial canvases across cores
            _scopes[-1].__exit__(None, None, None)
            _scopes.append(nc.named_scope("foldred"))
            _scopes[-1].__enter__()
            allreduce(mybir.AluOpType.add, cc5, cc6)

            # --------------------- normalize -------------------------------
            _scopes[-1].__exit__(None, None, None)
            _scopes.append(nc.named_scope("norm"))
            _scopes[-1].__enter__()
            crop_s = wpool.tile([H, C, W], f32)
            nc.sync.dma_start(crop_s[:], cc6[PAD:PAD + H, :, PAD:PAD + W])
            crop = crop_s[:]
            rowmax = wpool.tile([H, 1], f32)
            nc.vector.tensor_reduce(rowmax[:], crop,
                                    mybir.AxisListType.XY,
                                    mybir.AluOpType.max)
            drmax = dpool.tile([H, 1], f32)
            nc.sync.dma_start(drmax[:], rowmax[:])
            rmT = wpool.tile([1, H], f32)
            nc.sync.dma_start(rmT[:], drmax.transpose([1, 0])[:])
            gmax = wpool.tile([1, 1], f32)
            nc.vector.tensor_reduce(gmax[:], rmT[:],
                                    mybir.AxisListType.X,
                                    mybir.AluOpType.max)
            pb = ppool.tile([H, 1], f32)
            nc.tensor.matmul(pb[:], tones[:], gmax[:], start=True, stop=True)
            gmb = wpool.tile([H, 1], f32)
            nc.vector.tensor_copy(gmb[:], pb[:])
            rcp = wpool.tile([H, 1], f32)
            nc.vector.reciprocal(rcp[:], gmb[:])
            outn = wpool.tile([H, C, W], f32)
            nc.vector.tensor_scalar(outn[:], crop, rcp[:, 0:1], None,
                                    op0=mybir.AluOpType.mult)
            nc.sync.dma_start(out_d[:], outn[:])
            _scopes[-1].__exit__(None, None, None)

    nc.compile()
    return nc


def _get_nc(debug_gi=False):
    key = ("v3", debug_gi)
    if key not in _NC_CACHE:
        _NC_CACHE[key] = _build(debug_gi=debug_gi)
    return _NC_CACHE[key]


def _im2col(image):
    img = np.ascontiguousarray(image.transpose(2, 0, 1)).astype(np.float32)
    xp = np.pad(img, ((0, 0), (PAD, PAD), (PAD, PAD)))
    win = np.arange(OH)[:, None] + np.arange(K)[None, :]
    p = xp[:, win[:, None, :, None], win[None, :, None, :]]
    return p.transpose(1, 2, 0, 3, 4).reshape(L, D)


def _to6(x, rows, tiles, width):
    # (rows, D) -> (tiles, 128, DSTEP, width) lhsT/rhs layout
    return np.ascontiguousarray(
        x.T.reshape(DSTEP, 128, tiles, width).transpose(2, 1, 0, 3))


def _prepare_in_maps(image, mem_keys, mem_values, mode=None):
    q = _im2col(image)
    qpad = np.zeros((LP, D), dtype=np.float32)
    qpad[:L] = q
    qh = qpad.astype(ml_dtypes.bfloat16)
    ql = (qpad - qh.astype(np.float32)).astype(ml_dtypes.bfloat16)
    qh6 = _to6(qh, LP, MT, 128)
    ql6 = _to6(ql, LP, MT, 128)

    eyes = np.zeros((OH, K, 84), dtype=ml_dtypes.bfloat16)
    oh_i = np.arange(OH)
    for ky in range(K):
        eyes[oh_i, ky, oh_i + ky] = 1.0

    in_maps = []
    for c in range(NCORES):
        kc = mem_keys[c * NK:(c + 1) * NK]
        kcp = np.zeros((NKP, D), dtype=np.float32)
        kcp[:NK] = kc
        kh = kcp.astype(ml_dtypes.bfloat16)
        kl = (kcp - kh.astype(np.float32)).astype(ml_dtypes.bfloat16)

        bb = np.empty(NB, dtype=ml_dtypes.bfloat16)
        vc = mem_values[c * NK:(c + 1) * NK].astype(ml_dtypes.bfloat16)
        bb[OV:OV + NK * D] = vc.reshape(-1)
        bb[OV + NK * D:OV + SZ_VALS] = 0          # zero row
        bb[OE:OE + SZ_EYES] = eyes.reshape(-1)
        bb[OQH:OQH + SZ_Q] = qh6.reshape(-1)
        bb[OQL:OQL + SZ_Q] = ql6.reshape(-1)
        bb[OKH:OKH + SZ_K] = _to6(kh, NKP, NSUB, SUBW).reshape(-1)
        bb[OKL:OKL + SZ_K] = _to6(kl, NKP, NSUB, SUBW).reshape(-1)

        fb = np.empty(NF, dtype=np.float32)
        n2 = (kc.astype(np.float64) ** 2).sum(axis=1)
        bias = np.full(NKP, -1.0e9, dtype=np.float32)
        bias[:NK] = (-0.5 * n2).astype(np.float32)
        fb[OB:OB + 128 * NKP] = np.broadcast_to(bias, (128, NKP)).reshape(-1)
        bases = (c * NK + np.arange(NSUB, dtype=np.float32) * SUBW)
        fb[OBA:OBA + 128 * NSUB] = np.broadcast_to(
            bases, (128, NSUB)).reshape(-1)
        fb[OO:OO + 64] = 1.0
        bb[OF:NB] = fb.view(ml_dtypes.bfloat16)

        in_maps.append({"bb": bb})
    return in_maps


def kernel(**inputs):
    global LAST_EXEC_NS, LAST_RESULTS
    image = np.asarray(inputs["image"], dtype=np.float32)
    mem_keys = np.asarray(inputs["mem_keys"], dtype=np.float32)
    mem_values = np.asarray(inputs["mem_values"], dtype=np.float32)

    nc = _get_nc(debug_gi=DEBUG_GI)
    in_maps = _prepare_in_maps(image, mem_keys, mem_values)

    r = run_bass_kernel_spmd(nc, in_maps, list(range(NCORES)), trace=TRACE)
    LAST_EXEC_NS = r.exec_time_ns
    LAST_RESULTS = r.results
    out = r.results[0]["out"]
    return np.ascontiguousarray(out.transpose(0, 2, 1)).astype(np.float32)


if __name__ == "__main__":
    rng = np.random.default_rng(0)
    ins = dict(
        image=rng.random((H, W, C), dtype=np.float32),
        mem_keys=rng.standard_normal((N_MEM, D), dtype=np.float32),
        mem_values=rng.standard_normal((N_MEM, D), dtype=np.float32),
    )
    o = kernel(**ins)
    print("out", o.shape, o.dtype, float(o.max()), float(o.min()))


# revision 19
# speedup vs baseline: 1.1681x; 1.1681x over previous
import sys
import os

sys.path.insert(0, "/opt/trn_rl_repo")

import numpy as np
import ml_dtypes

import concourse.bacc as bacc
import concourse.bass as bass
import concourse.tile as tile
from concourse import mybir
from concourse.bass_utils import run_bass_kernel_spmd

f32 = mybir.dt.float32
bf16 = mybir.dt.bfloat16
u32 = mybir.dt.uint32
i32 = mybir.dt.int32

# problem geometry (hardcoded; kernel.py must be self-contained)
H = W = 64
C = 3
K = 32
PAD = 10
OH = H + 2 * PAD - K + 1          # 53
L = OH * OH                       # 2809
MT = 22                           # m-tiles of 128 rows: 22*128 = 2816 >= L
LP = MT * 128
D = C * K * K                     # 3072
DSTEP = D // 128                  # 24
N_MEM = 20000
NCORES = 8
NK = N_MEM // NCORES              # 2500 keys per core
NSUB = 5                          # key column tiles of 512
SUBW = 512
NKP = NSUB * SUBW                 # 2560 (padded)
VCH = 3                           # patch columns folded per chunk

# ---- packed input blob layout (bf16 element offsets) ----
# [vals | eyes | qh | ql | kh | kl | f32 tail (bitcast): bias | bases | ones]
SZ_VALS = (NK + 1) * D            # 7,683,072  (vals first: indirect src @ 0)
SZ_EYES = OH * K * 84
SZ_Q1 = 128 * DSTEP * 128         # one m-tile of queries
SZ_Q = MT * SZ_Q1
SZ_K1 = 128 * DSTEP * SUBW        # one si-tile of keys
SZ_K = NSUB * SZ_K1
OV = 0
OE = OV + SZ_VALS
OQH = OE + SZ_EYES
OQL = OQH + SZ_Q
OKH = OQL + SZ_Q
OKL = OKH + SZ_K
OF = OKL + SZ_K                   # f32 tail starts here (even offset)
# f32-tail offsets, in f32 elements relative to OF
OB = 0
OBA = OB + 128 * NKP
OO = OBA + 128 * NSUB
NF = OO + 64
NB = OF + 2 * NF

GROUPS = [[0, 1], [2, 3], [4]]

TRACE = False
DEBUG_GI = False                  # adds a 'gi' debug output (costly on wall)
LAST_EXEC_NS = None
LAST_RESULTS = None

_NC_CACHE = {}


def _build(collectives=True, debug_gi=False):
    nc = bacc.Bacc("TRN2", target_bir_lowering=False, debug=False,
                   num_devices=NCORES)

    def allreduce(op, src, dst):
        if collectives:
            nc.gpsimd.collective_compute(
                "AllReduce", op,
                replica_groups=[list(range(NCORES))],
                ins=[src.opt()], outs=[dst.opt()])
        else:
            nc.sync.dma_start(dst[:], src[:])

    bb_d = nc.dram_tensor("bb", [NB], bf16, kind="ExternalInput")
    out_d = nc.dram_tensor("out", [64, C, 64], f32, kind="ExternalOutput")
    if debug_gi:
        gi_d = nc.dram_tensor("gi", [128, MT], f32, kind="ExternalOutput")

    def bview(ofs, sz, pat, **kw):
        return bb_d[ofs:ofs + sz].rearrange(pat, **kw)

    def fview(ofs, sz, pat, **kw):
        return bb_d[OF + 2 * ofs:OF + 2 * (ofs + sz)].bitcast(f32).rearrange(
            pat, **kw)

    with tile.TileContext(nc) as tc:
        with (
            tc.tile_pool(name="keys", bufs=1) as kpool,
            tc.tile_pool(name="qp", bufs=2) as qpool,
            tc.tile_pool(name="work", bufs=1) as wpool,
            tc.tile_pool(name="sm", bufs=2) as mpool,
            tc.tile_pool(name="vt", bufs=2) as vpool,
            tc.tile_pool(name="psum", bufs=2, space=bass.MemorySpace.PSUM) as ppool,
            tc.tile_pool(name="dram", bufs=1, space="DRAM") as dpool,
        ):
            tbias = wpool.tile([128, NKP], f32)
            nc.sync.dma_start(
                tbias[:], fview(OB, 128 * NKP, "(p c) -> p c", p=128, c=NKP))
            tbases = wpool.tile([128, NSUB], f32)
            nc.sync.dma_start(
                tbases[:], fview(OBA, 128 * NSUB, "(p c) -> p c",
                                 p=128, c=NSUB))
            tones = wpool.tile([1, 64], f32)
            nc.sync.dma_start(
                tones[:], fview(OO, 64, "(p c) -> p c", p=1, c=64))
            eyes = wpool.tile([OH, K, 84], bf16)
            nc.sync.dma_start(
                eyes[:], bview(OE, SZ_EYES, "(a k y) -> a k y",
                               a=OH, k=K, y=84))

            best = wpool.tile([128, MT], f32)
            bix = wpool.tile([128, MT], f32)

            # ---------------- scan: scores + per-core argmax ----------------
            # si-outer with double-buffered per-si key tiles: the next si's
            # key load overlaps this si's 22 m-tile matmul chains.
            _scopes = [nc.named_scope("scan")]
            _scopes[-1].__enter__()
            for s in range(NSUB):
                kht = kpool.tile([128, DSTEP, SUBW], bf16, bufs=2)
                klt = kpool.tile([128, DSTEP, SUBW], bf16, bufs=2)
                nc.sync.dma_start(
                    kht[:],
                    bview(OKH + s * SZ_K1, SZ_K1, "(p d w) -> p d w",
                          p=128, d=DSTEP, w=SUBW))
                nc.sync.dma_start(
                    klt[:],
                    bview(OKL + s * SZ_K1, SZ_K1, "(p d w) -> p d w",
                          p=128, d=DSTEP, w=SUBW))

                for m in range(MT):
                    qht = qpool.tile([128, DSTEP, 128], bf16)
                    qlt = qpool.tile([128, DSTEP, 128], bf16)
                    nc.sync.dma_start(
                        qht[:], bview(OQH + m * SZ_Q1, SZ_Q1, "(p d w) -> p d w",
                                      p=128, d=DSTEP, w=128))
                    nc.sync.dma_start(
                        qlt[:], bview(OQL + m * SZ_Q1, SZ_Q1, "(p d w) -> p d w",
                                      p=128, d=DSTEP, w=128))

                    acc = ppool.tile([128, SUBW], f32)
                    passes = [(qht, kht), (qht, klt), (qlt, kht)]
                    nmm = DSTEP * len(passes)
                    i = 0
                    for d in range(DSTEP):
                        for (lt, rt) in passes:
                            nc.tensor.matmul(acc[:], lt[:, d, :],
                                             rt[:, d, :],
                                             start=(i == 0),
                                             stop=(i == nmm - 1))
                            i += 1

                    sc = mpool.tile([128, SUBW], f32)
                    nc.vector.scalar_tensor_tensor(
                        sc[:], acc[:], 1.0,
                        tbias[:, s * SUBW:(s + 1) * SUBW],
                        op0=mybir.AluOpType.mult,
                        op1=mybir.AluOpType.add)
                    mxv = mpool.tile([128, 8], f32)
                    mxi = mpool.tile([128, 8], u32)
                    nc.vector.max_with_indices(mxv[:], mxi[:], sc[:])
                    nixf = mpool.tile([128, 1], f32)
                    nc.vector.tensor_copy(nixf[:], mxi[:, 0:1])
                    nix2 = mpool.tile([128, 1], f32)
                    nc.vector.tensor_scalar_add(nix2[:], nixf[:],
                                                tbases[:, s:s + 1])
                    if s == 0:
                        nc.vector.tensor_copy(best[:, m:m + 1],
                                              mxv[:, 0:1])
                        nc.vector.tensor_copy(bix[:, m:m + 1], nix2[:])
                    else:
                        gt = mpool.tile([128, 1], u32)
                        nc.vector.scalar_tensor_tensor(
                            gt[:], mxv[:, 0:1], 1.0, best[:, m:m + 1],
                            op0=mybir.AluOpType.mult,
                            op1=mybir.AluOpType.is_gt)
                        nc.vector.copy_predicated(best[:, m:m + 1],
                                                  gt[:], mxv[:, 0:1])
                        nc.vector.copy_predicated(bix[:, m:m + 1],
                                                  gt[:], nix2[:])

            # ------------- global argmin via AllReduce(max)+(min) -----------
            _scopes[-1].__exit__(None, None, None)
            _scopes.append(nc.named_scope("argminred"))
            _scopes[-1].__enter__()
            cc1 = dpool.tile([128, MT], f32)
            cc2 = dpool.tile([128, MT], f32)
            nc.gpsimd.dma_start(cc1[:], best[:])
            allreduce(mybir.AluOpType.max, cc1, cc2)
            gbest = wpool.tile([128, MT], f32)
            nc.gpsimd.dma_start(gbest[:], cc2[:])

            ge = wpool.tile([128, MT], f32)
            nc.vector.scalar_tensor_tensor(
                ge[:], best[:], 1.0, gbest[:],
                op0=mybir.AluOpType.mult, op1=mybir.AluOpType.is_ge)
            t1 = wpool.tile([128, MT], f32)
            nc.vector.tensor_scalar_add(t1[:], bix[:], -1.0e6)
            t2 = wpool.tile([128, MT], f32)
            nc.vector.scalar_tensor_tensor(
                t2[:], ge[:], 1.0, t1[:],
                op0=mybir.AluOpType.mult, op1=mybir.AluOpType.mult)
            cand = wpool.tile([128, MT], f32)
            nc.vector.tensor_scalar_add(cand[:], t2[:], 1.0e6)

            cc3 = dpool.tile([128, MT], f32)
            cc4 = dpool.tile([128, MT], f32)
            nc.gpsimd.dma_start(cc3[:], cand[:])
            allreduce(mybir.AluOpType.min, cc3, cc4)
            gif = wpool.tile([128, MT], f32)
            nc.gpsimd.dma_start(gif[:], cc4[:])
            if debug_gi:
                nc.sync.dma_start(gi_d[:], gif[:])

            # local row index: owned -> gi - c*2500, else zero row NK;
            # scaled by D to give a flat bf16-blob element offset.
            li = wpool.tile([128, MT], f32)
            nc.vector.tensor_scalar(li[:], gif[:], tbases[:, 0:1], None,
                                    op0=mybir.AluOpType.subtract)
            o1 = wpool.tile([128, MT], f32)
            nc.vector.tensor_scalar(o1[:], li[:], 0.0, None,
                                    op0=mybir.AluOpType.is_ge)
            o2 = wpool.tile([128, MT], f32)
            nc.vector.tensor_scalar(o2[:], li[:], float(NK), None,
                                    op0=mybir.AluOpType.is_lt)
            own = wpool.tile([128, MT], f32)
            nc.vector.scalar_tensor_tensor(
                own[:], o1[:], 1.0, o2[:],
                op0=mybir.AluOpType.mult, op1=mybir.AluOpType.mult)
            d1 = wpool.tile([128, MT], f32)
            nc.vector.tensor_scalar_add(d1[:], li[:], -float(NK))
            d2t = wpool.tile([128, MT], f32)
            nc.vector.scalar_tensor_tensor(
                d2t[:], own[:], 1.0, d1[:],
                op0=mybir.AluOpType.mult, op1=mybir.AluOpType.mult)
            lc = wpool.tile([128, MT], f32)
            nc.vector.tensor_scalar_add(lc[:], d2t[:], float(NK))
            lci = wpool.tile([128, MT], i32)
            nc.vector.tensor_copy(lci[:], lc[:])

            # relayout [128, MT] -> [1, LP] (patch-id order) -> [oh, ow] grid
            gidr = dpool.tile([128, MT], i32)
            nc.sync.dma_start(gidr[:], lci[:])
            gi32 = wpool.tile([1, LP], i32)
            nc.sync.dma_start(gi32[:], gidr.transpose([1, 0])[:])
            dgrid = dpool.tile([OH, OH], i32)
            nc.sync.dma_start(
                dgrid[:], gi32[0:1, 0:L].rearrange("p (a b) -> p a b",
                                                   a=OH, b=OH))
            idxT = wpool.tile([OH, OH], i32)
            nc.sync.dma_start(idxT[:], dgrid[:])

            # --------------------- gather + fold ---------------------------
            _scopes[-1].__exit__(None, None, None)
            _scopes.append(nc.named_scope("gatherfold"))
            _scopes[-1].__enter__()

            vtab = bb_d[0:SZ_VALS].rearrange("(r d) -> r d", r=NK + 1, d=D)
            Wt = wpool.tile([84, OH, C, K], bf16)
            c0 = 0
            while c0 < OH:
                clen = min(VCH, OH - c0)
                vtc = vpool.tile([128, VCH, D], bf16)
                for j in range(clen):
                    nc.gpsimd.indirect_dma_start(
                        out=vtc[0:OH, j, :],
                        out_offset=None,
                        in_=vtab,
                        in_offset=bass.IndirectOffsetOnAxis(
                            ap=idxT[0:OH, c0 + j:c0 + j + 1], axis=0),
                    )
                vtR = vtc[:].rearrange("q g (c ky kx) -> q g c ky kx",
                                       c=C, ky=K, kx=K)
                zp = ppool.tile([84, VCH * C * K], f32)
                for ky in range(K):
                    nc.tensor.matmul(zp[:, 0:clen * C * K],
                                     eyes[0:53, ky, :],
                                     vtR[0:53, 0:clen, :, ky, :],
                                     start=(ky == 0), stop=(ky == K - 1))
                nc.vector.tensor_copy(
                    Wt[0:84, c0:c0 + clen, :, :],
                    zp[:, 0:clen * C * K].rearrange(
                        "p (g c k) -> p g c k", g=clen, c=C, k=K))
                c0 += clen

            # stage B: fold along ow via strided in-place adds
            cl = wpool.tile([84, C, 84], f32)
            nc.vector.memset(cl[:], 0.0)
            for kx in range(K):
                nc.vector.scalar_tensor_tensor(
                    cl[0:84, :, kx:kx + OH],
                    Wt[0:84, :, :, kx].transpose([0, 2, 1]), 1.0,
                    cl[0:84, :, kx:kx + OH],
                    op0=mybir.AluOpType.mult, op1=mybir.AluOpType.add)

            cc5 = dpool.tile([H + 2 * PAD, C, H + 2 * PAD], f32)
            cc6 = dpool.tile([H + 2 * PAD, C, H + 2 * PAD], f32)
            nc.sync.dma_start(cc5[:], cl[:])

            # sum partial canvases across cores
            _scopes[-1].__exit__(None, None, None)
            _scopes.append(nc.named_scope("foldred"))
            _scopes[-1].__enter__()
            allreduce(mybir.AluOpType.add, cc5, cc6)

            # --------------------- normalize -------------------------------
            _scopes[-1].__exit__(None, None, None)
            _scopes.append(nc.named_scope("norm"))
            _scopes[-1].__enter__()
            crop_s = wpool.tile([H, C, W], f32)
            nc.sync.dma_start(crop_s[:], cc6[PAD:PAD + H, :, PAD:PAD + W])
            crop = crop_s[:]
            rowmax = wpool.tile([H, 1], f32)
            nc.vector.tensor_reduce(rowmax[:], crop,
                                    mybir.AxisListType.XY,
                                    mybir.AluOpType.max)
            drmax = dpool.tile([H, 1], f32)
            nc.sync.dma_start(drmax[:], rowmax[:])
            rmT = wpool.tile([1, H], f32)
            nc.sync.dma_start(rmT[:], drmax.transpose([1, 0])[:])
            gmax = wpool.tile([1, 1], f32)
            nc.vector.tensor_reduce(gmax[:], rmT[:],
                                    mybir.AxisListType.X,
                                    mybir.AluOpType.max)
            pb = ppool.tile([H, 1], f32)
            nc.tensor.matmul(pb[:], tones[:], gmax[:], start=True, stop=True)
            gmb = wpool.tile([H, 1], f32)
            nc.vector.tensor_copy(gmb[:], pb[:])
            rcp = wpool.tile([H, 1], f32)
            nc.vector.reciprocal(rcp[:], gmb[:])
            outn = wpool.tile([H, C, W], f32)
            nc.vector.tensor_scalar(outn[:], crop, rcp[:, 0:1], None,
                                    op0=mybir.AluOpType.mult)
            nc.sync.dma_start(out_d[:], outn[:])
            _scopes[-1].__exit__(None, None, None)

    nc.compile()
    return nc


def _get_nc(debug_gi=False):
    key = ("v3", debug_gi)
    if key not in _NC_CACHE:
        _NC_CACHE[key] = _build(debug_gi=debug_gi)
    return _NC_CACHE[key]


def _im2col(image):
    img = np.ascontiguousarray(image.transpose(2, 0, 1)).astype(np.float32)
    xp = np.pad(img, ((0, 0), (PAD, PAD), (PAD, PAD)))
    win = np.arange(OH)[:, None] + np.arange(K)[None, :]
    p = xp[:, win[:, None, :, None], win[None, :, None, :]]
    return p.transpose(1, 2, 0, 3, 4).reshape(L, D)


def _to6(x, rows, tiles, width):
    # (rows, D) -> (tiles, 128, DSTEP, width) lhsT/rhs layout
    return np.ascontiguousarray(
        x.T.reshape(DSTEP, 128, tiles, width).transpose(2, 1, 0, 3))


def _prepare_in_maps(image, mem_keys, mem_values, mode=None):
    q = _im2col(image)
    qpad = np.zeros((LP, D), dtype=np.float32)
    qpad[:L] = q
    qh = qpad.astype(ml_dtypes.bfloat16)
    ql = (qpad - qh.astype(np.float32)).astype(ml_dtypes.bfloat16)
    qh6 = _to6(qh, LP, MT, 128)
    ql6 = _to6(ql, LP, MT, 128)

    eyes = np.zeros((OH, K, 84), dtype=ml_dtypes.bfloat16)
    oh_i = np.arange(OH)
    for ky in range(K):
        eyes[oh_i, ky, oh_i + ky] = 1.0

    in_maps = []
    for c in range(NCORES):
        kc = mem_keys[c * NK:(c + 1) * NK]
        kcp = np.zeros((NKP, D), dtype=np.float32)
        kcp[:NK] = kc
        kh = kcp.astype(ml_dtypes.bfloat16)
        kl = (kcp - kh.astype(np.float32)).astype(ml_dtypes.bfloat16)

        bb = np.empty(NB, dtype=ml_dtypes.bfloat16)
        vc = mem_values[c * NK:(c + 1) * NK].astype(ml_dtypes.bfloat16)
        bb[OV:OV + NK * D] = vc.reshape(-1)
        bb[OV + NK * D:OV + SZ_VALS] = 0          # zero row
        bb[OE:OE + SZ_EYES] = eyes.reshape(-1)
        bb[OQH:OQH + SZ_Q] = qh6.reshape(-1)
        bb[OQL:OQL + SZ_Q] = ql6.reshape(-1)
        bb[OKH:OKH + SZ_K] = _to6(kh, NKP, NSUB, SUBW).reshape(-1)
        bb[OKL:OKL + SZ_K] = _to6(kl, NKP, NSUB, SUBW).reshape(-1)

        fb = np.empty(NF, dtype=np.float32)
        n2 = (kc.astype(np.float64) ** 2).sum(axis=1)
        bias = np.full(NKP, -1.0e9, dtype=np.float32)
        bias[:NK] = (-0.5 * n2).astype(np.float32)
        fb[OB:OB + 128 * NKP] = np.broadcast_to(bias, (128, NKP)).reshape(-1)
        bases = (c * NK + np.arange(NSUB, dtype=np.float32) * SUBW)
        fb[OBA:OBA + 128 * NSUB] = np.broadcast_to(
            bases, (128, NSUB)).reshape(-1)
        fb[OO:OO + 64] = 1.0
        bb[OF:NB] = fb.view(ml_dtypes.bfloat16)

        in_maps.append({"bb": bb})
    return in_maps


def kernel(**inputs):
    global LAST_EXEC_NS, LAST_RESULTS
    image = np.asarray(inputs["image"], dtype=np.float32)
    mem_keys = np.asarray(inputs["mem_keys"], dtype=np.float32)
    mem_values = np.asarray(inputs["mem_values"], dtype=np.float32)

    nc = _get_nc(debug_gi=DEBUG_GI)
    in_maps = _prepare_in_maps(image, mem_keys, mem_values)

    r = run_bass_kernel_spmd(nc, in_maps, list(range(NCORES)), trace=TRACE)
    LAST_EXEC_NS = r.exec_time_ns
    LAST_RESULTS = r.results
    out = r.results[0]["out"]
    return np.ascontiguousarray(out.transpose(0, 2, 1)).astype(np.float32)


if __name__ == "__main__":
    rng = np.random.default_rng(0)
    ins = dict(
        image=rng.random((H, W, C), dtype=np.float32),
        mem_keys=rng.standard_normal((N_MEM, D), dtype=np.float32),
        mem_values=rng.standard_normal((N_MEM, D), dtype=np.float32),
    )
    o = kernel(**ins)
    print("out", o.shape, o.dtype, float(o.max()), float(o.min()))


# revision 26
# speedup vs baseline: 1.1986x; 1.0262x over previous
import sys
import os

sys.path.insert(0, "/opt/trn_rl_repo")

import numpy as np
import ml_dtypes

import concourse.bacc as bacc
import concourse.bass as bass
import concourse.tile as tile
from concourse import mybir
from concourse.bass_utils import run_bass_kernel_spmd

f32 = mybir.dt.float32
bf16 = mybir.dt.bfloat16
u32 = mybir.dt.uint32
i32 = mybir.dt.int32

# problem geometry (hardcoded; kernel.py must be self-contained)
H = W = 64
C = 3
K = 32
PAD = 10
OH = H + 2 * PAD - K + 1          # 53
L = OH * OH                       # 2809
MT = 22                           # m-tiles of 128 rows: 22*128 = 2816 >= L
LP = MT * 128
D = C * K * K                     # 3072
DSTEP = D // 128                  # 24
N_MEM = 20000
NCORES = 8
NK = N_MEM // NCORES              # 2500 keys per core
NSUB = 5                          # key column tiles of 512
SUBW = 512
NKP = NSUB * SUBW                 # 2560 (padded)
VCH = 3                           # patch columns folded per chunk

# ---- packed input blob layout (bf16 element offsets) ----
# [vals | eyes | qh | ql | kh | kl | f32 tail (bitcast): bias | bases | ones]
SZ_VALS = (NK + 1) * D            # 7,683,072  (vals first: indirect src @ 0)
SZ_EYES = OH * K * 84
SZ_Q1 = 128 * DSTEP * 128         # one m-tile of queries
SZ_Q = MT * SZ_Q1
SZ_K1 = 128 * DSTEP * SUBW        # one si-tile of keys
SZ_K = NSUB * SZ_K1
OV = 0
OE = OV + SZ_VALS
OQH = OE + SZ_EYES
OQL = OQH + SZ_Q
OKH = OQL + SZ_Q
OKL = OKH + SZ_K
OF = OKL + SZ_K                   # f32 tail starts here (even offset)
# f32-tail offsets, in f32 elements relative to OF
OB = 0
OBA = OB + 128 * NKP
OO = OBA + 128 * NSUB
NF = OO + 64
NB = OF + 2 * NF

GROUPS = [[0, 1], [2, 3], [4]]

TRACE = False
DEBUG_GI = False                  # adds a 'gi' debug output (costly on wall)
LAST_EXEC_NS = None
LAST_RESULTS = None

_NC_CACHE = {}


def _build(collectives=True, debug_gi=False):
    nc = bacc.Bacc("TRN2", target_bir_lowering=False, debug=False,
                   num_devices=NCORES)

    def allreduce(op, src, dst):
        if collectives:
            nc.gpsimd.collective_compute(
                "AllReduce", op,
                replica_groups=[list(range(NCORES))],
                ins=[src.opt()], outs=[dst.opt()])
        else:
            nc.sync.dma_start(dst[:], src[:])

    bb_d = nc.dram_tensor("bb", [NB], bf16, kind="ExternalInput")
    out_d = nc.dram_tensor("out", [64, C, 64], f32, kind="ExternalOutput")
    if debug_gi:
        gi_d = nc.dram_tensor("gi", [128, MT], f32, kind="ExternalOutput")

    def bview(ofs, sz, pat, **kw):
        return bb_d[ofs:ofs + sz].rearrange(pat, **kw)

    def fview(ofs, sz, pat, **kw):
        return bb_d[OF + 2 * ofs:OF + 2 * (ofs + sz)].bitcast(f32).rearrange(
            pat, **kw)

    with tile.TileContext(nc) as tc:
        with (
            tc.tile_pool(name="keys", bufs=1) as kpool,
            tc.tile_pool(name="qp", bufs=2) as qpool,
            tc.tile_pool(name="work", bufs=1) as wpool,
            tc.tile_pool(name="sm", bufs=2) as mpool,
            tc.tile_pool(name="vt", bufs=2) as vpool,
            tc.tile_pool(name="psum", bufs=2, space=bass.MemorySpace.PSUM) as ppool,
            tc.tile_pool(name="dram", bufs=1, space="DRAM") as dpool,
        ):
            # constants go on the scalar engine's DMA queue so they don't
            # head-of-line-block the first key/query loads on sync
            tbias = wpool.tile([128, NKP], f32)
            nc.scalar.dma_start(
                tbias[:], fview(OB, 128 * NKP, "(p c) -> p c", p=128, c=NKP))
            tbases = wpool.tile([128, NSUB], f32)
            nc.scalar.dma_start(
                tbases[:], fview(OBA, 128 * NSUB, "(p c) -> p c",
                                 p=128, c=NSUB))
            tones = wpool.tile([1, 64], f32)
            nc.scalar.dma_start(
                tones[:], fview(OO, 64, "(p c) -> p c", p=1, c=64))
            eyes = wpool.tile([OH, K, 84], bf16)
            nc.scalar.dma_start(
                eyes[:], bview(OE, SZ_EYES, "(a k y) -> a k y",
                               a=OH, k=K, y=84))

            best = wpool.tile([128, MT], f32)
            bix = wpool.tile([128, MT], f32)

            # ---------------- scan: scores + per-core argmax ----------------
            # si-outer with double-buffered per-si key tiles: the next si's
            # key load overlaps this si's 22 m-tile matmul chains.
            _scopes = [nc.named_scope("scan")]
            _scopes[-1].__enter__()
            for s in range(NSUB):
                kht = kpool.tile([128, DSTEP, SUBW], bf16, bufs=2)
                klt = kpool.tile([128, DSTEP, SUBW], bf16, bufs=2)
                nc.sync.dma_start(
                    kht[:],
                    bview(OKH + s * SZ_K1, SZ_K1, "(p d w) -> p d w",
                          p=128, d=DSTEP, w=SUBW))
                nc.sync.dma_start(
                    klt[:],
                    bview(OKL + s * SZ_K1, SZ_K1, "(p d w) -> p d w",
                          p=128, d=DSTEP, w=SUBW))

                for m in range(MT):
                    qht = qpool.tile([128, DSTEP, 128], bf16)
                    qlt = qpool.tile([128, DSTEP, 128], bf16)
                    nc.sync.dma_start(
                        qht[:], bview(OQH + m * SZ_Q1, SZ_Q1, "(p d w) -> p d w",
                                      p=128, d=DSTEP, w=128))
                    nc.sync.dma_start(
                        qlt[:], bview(OQL + m * SZ_Q1, SZ_Q1, "(p d w) -> p d w",
                                      p=128, d=DSTEP, w=128))

                    acc = ppool.tile([128, SUBW], f32)
                    passes = [(qht, kht), (qht, klt), (qlt, kht)]
                    nmm = DSTEP * len(passes)
                    i = 0
                    for d in range(DSTEP):
                        for (lt, rt) in passes:
                            nc.tensor.matmul(acc[:], lt[:, d, :],
                                             rt[:, d, :],
                                             start=(i == 0),
                                             stop=(i == nmm - 1))
                            i += 1

                    sc = mpool.tile([128, SUBW], f32)
                    nc.vector.scalar_tensor_tensor(
                        sc[:], acc[:], 1.0,
                        tbias[:, s * SUBW:(s + 1) * SUBW],
                        op0=mybir.AluOpType.mult,
                        op1=mybir.AluOpType.add)
                    mxv = mpool.tile([128, 8], f32)
                    mxi = mpool.tile([128, 8], u32)
                    nc.vector.max_with_indices(mxv[:], mxi[:], sc[:])
                    nixf = mpool.tile([128, 1], f32)
                    nc.vector.tensor_copy(nixf[:], mxi[:, 0:1])
                    nix2 = mpool.tile([128, 1], f32)
                    nc.vector.tensor_scalar_add(nix2[:], nixf[:],
                                                tbases[:, s:s + 1])
                    if s == 0:
                        nc.vector.tensor_copy(best[:, m:m + 1],
                                              mxv[:, 0:1])
                        nc.vector.tensor_copy(bix[:, m:m + 1], nix2[:])
                    else:
                        gt = mpool.tile([128, 1], u32)
                        nc.vector.scalar_tensor_tensor(
                            gt[:], mxv[:, 0:1], 1.0, best[:, m:m + 1],
                            op0=mybir.AluOpType.mult,
                            op1=mybir.AluOpType.is_gt)
                        nc.vector.copy_predicated(best[:, m:m + 1],
                                                  gt[:], mxv[:, 0:1])
                        nc.vector.copy_predicated(bix[:, m:m + 1],
                                                  gt[:], nix2[:])

            # ------------- global argmin via AllReduce(max)+(min) -----------
            _scopes[-1].__exit__(None, None, None)
            _scopes.append(nc.named_scope("argminred"))
            _scopes[-1].__enter__()
            cc1 = dpool.tile([128, MT], f32)
            cc2 = dpool.tile([128, MT], f32)
            nc.gpsimd.dma_start(cc1[:], best[:])
            allreduce(mybir.AluOpType.max, cc1, cc2)
            gbest = wpool.tile([128, MT], f32)
            nc.gpsimd.dma_start(gbest[:], cc2[:])

            ge = wpool.tile([128, MT], f32)
            nc.vector.scalar_tensor_tensor(
                ge[:], best[:], 1.0, gbest[:],
                op0=mybir.AluOpType.mult, op1=mybir.AluOpType.is_ge)
            t1 = wpool.tile([128, MT], f32)
            nc.vector.tensor_scalar_add(t1[:], bix[:], -1.0e6)
            t2 = wpool.tile([128, MT], f32)
            nc.vector.scalar_tensor_tensor(
                t2[:], ge[:], 1.0, t1[:],
                op0=mybir.AluOpType.mult, op1=mybir.AluOpType.mult)
            cand = wpool.tile([128, MT], f32)
            nc.vector.tensor_scalar_add(cand[:], t2[:], 1.0e6)

            cc3 = dpool.tile([128, MT], f32)
            cc4 = dpool.tile([128, MT], f32)
            nc.gpsimd.dma_start(cc3[:], cand[:])
            allreduce(mybir.AluOpType.min, cc3, cc4)
            gif = wpool.tile([128, MT], f32)
            nc.gpsimd.dma_start(gif[:], cc4[:])
            if debug_gi:
                nc.sync.dma_start(gi_d[:], gif[:])

            # local row index: owned -> gi - c*2500, else zero row NK;
            # scaled by D to give a flat bf16-blob element offset.
            li = wpool.tile([128, MT], f32)
            nc.vector.tensor_scalar(li[:], gif[:], tbases[:, 0:1], None,
                                    op0=mybir.AluOpType.subtract)
            o1 = wpool.tile([128, MT], f32)
            nc.vector.tensor_scalar(o1[:], li[:], 0.0, None,
                                    op0=mybir.AluOpType.is_ge)
            o2 = wpool.tile([128, MT], f32)
            nc.vector.tensor_scalar(o2[:], li[:], float(NK), None,
                                    op0=mybir.AluOpType.is_lt)
            own = wpool.tile([128, MT], f32)
            nc.vector.scalar_tensor_tensor(
                own[:], o1[:], 1.0, o2[:],
                op0=mybir.AluOpType.mult, op1=mybir.AluOpType.mult)
            d1 = wpool.tile([128, MT], f32)
            nc.vector.tensor_scalar_add(d1[:], li[:], -float(NK))
            d2t = wpool.tile([128, MT], f32)
            nc.vector.scalar_tensor_tensor(
                d2t[:], own[:], 1.0, d1[:],
                op0=mybir.AluOpType.mult, op1=mybir.AluOpType.mult)
            lc = wpool.tile([128, MT], f32)
            nc.vector.tensor_scalar_add(lc[:], d2t[:], float(NK))
            lci = wpool.tile([128, MT], i32)
            nc.vector.tensor_copy(lci[:], lc[:])

            # relayout [128, MT] -> [1, LP] (patch-id order) -> [oh, ow] grid
            gidr = dpool.tile([128, MT], i32)
            nc.sync.dma_start(gidr[:], lci[:])
            gi32 = wpool.tile([1, LP], i32)
            nc.sync.dma_start(gi32[:], gidr.transpose([1, 0])[:])
            idxT = wpool.tile([OH, OH], i32)
            nc.sync.dma_start(
                idxT[:], gi32[0:1, 0:L].rearrange("p (a b) -> p a b",
                                                  a=OH, b=OH))

            # --------------------- gather + fold ---------------------------
            _scopes[-1].__exit__(None, None, None)
            _scopes.append(nc.named_scope("gatherfold"))
            _scopes[-1].__enter__()

            vtab = bb_d[0:SZ_VALS].rearrange("(r d) -> r d", r=NK + 1, d=D)
            Wt = wpool.tile([84, OH, C, K], bf16)
            c0 = 0
            while c0 < OH:
                clen = min(VCH, OH - c0)
                vtc = vpool.tile([128, VCH, D], bf16)
                for j in range(clen):
                    nc.gpsimd.indirect_dma_start(
                        out=vtc[0:OH, j, :],
                        out_offset=None,
                        in_=vtab,
                        in_offset=bass.IndirectOffsetOnAxis(
                            ap=idxT[0:OH, c0 + j:c0 + j + 1], axis=0),
                    )
                vtR = vtc[:].rearrange("q g (c ky kx) -> q g c ky kx",
                                       c=C, ky=K, kx=K)
                zp = ppool.tile([84, VCH * C * K], f32)
                for ky in range(K):
                    nc.tensor.matmul(zp[:, 0:clen * C * K],
                                     eyes[0:53, ky, :],
                                     vtR[0:53, 0:clen, :, ky, :],
                                     start=(ky == 0), stop=(ky == K - 1))
                nc.vector.tensor_copy(
                    Wt[0:84, c0:c0 + clen, :, :],
                    zp[:, 0:clen * C * K].rearrange(
                        "p (g c k) -> p g c k", g=clen, c=C, k=K))
                c0 += clen

            # stage B: fold along ow via strided in-place adds
            cl = wpool.tile([84, C, 84], f32)
            nc.vector.memset(cl[:], 0.0)
            for kx in range(K):
                nc.vector.scalar_tensor_tensor(
                    cl[0:84, :, kx:kx + OH],
                    Wt[0:84, :, :, kx].transpose([0, 2, 1]), 1.0,
                    cl[0:84, :, kx:kx + OH],
                    op0=mybir.AluOpType.mult, op1=mybir.AluOpType.add)

            cc5 = dpool.tile([H + 2 * PAD, C, H + 2 * PAD], f32)
            cc6 = dpool.tile([H + 2 * PAD, C, H + 2 * PAD], f32)
            nc.sync.dma_start(cc5[:], cl[:])

            # sum partial canvases across cores
            _scopes[-1].__exit__(None, None, None)
            _scopes.append(nc.named_scope("foldred"))
            _scopes[-1].__enter__()
            allreduce(mybir.AluOpType.add, cc5, cc6)

            # --------------------- normalize -------------------------------
            _scopes[-1].__exit__(None, None, None)
            _scopes.append(nc.named_scope("norm"))
            _scopes[-1].__enter__()
            crop_s = wpool.tile([H, C, W], f32)
            nc.sync.dma_start(crop_s[:], cc6[PAD:PAD + H, :, PAD:PAD + W])
            crop = crop_s[:]
            rowmax = wpool.tile([H, 1], f32)
            nc.vector.tensor_reduce(rowmax[:], crop,
                                    mybir.AxisListType.XY,
                                    mybir.AluOpType.max)
            drmax = dpool.tile([H, 1], f32)
            nc.sync.dma_start(drmax[:], rowmax[:])
            rmT = wpool.tile([1, H], f32)
            nc.sync.dma_start(rmT[:], drmax.transpose([1, 0])[:])
            gmax = wpool.tile([1, 1], f32)
            nc.vector.tensor_reduce(gmax[:], rmT[:],
                                    mybir.AxisListType.X,
                                    mybir.AluOpType.max)
            pb = ppool.tile([H, 1], f32)
            nc.tensor.matmul(pb[:], tones[:], gmax[:], start=True, stop=True)
            gmb = wpool.tile([H, 1], f32)
            nc.vector.tensor_copy(gmb[:], pb[:])
            rcp = wpool.tile([H, 1], f32)
            nc.vector.reciprocal(rcp[:], gmb[:])
            outn = wpool.tile([H, C, W], f32)
            nc.vector.tensor_scalar(outn[:], crop, rcp[:, 0:1], None,
                                    op0=mybir.AluOpType.mult)
            nc.sync.dma_start(out_d[:], outn[:])
            _scopes[-1].__exit__(None, None, None)

    nc.compile()
    return nc


def _get_nc(debug_gi=False):
    key = ("v3", debug_gi)
    if key not in _NC_CACHE:
        _NC_CACHE[key] = _build(debug_gi=debug_gi)
    return _NC_CACHE[key]


def _im2col(image):
    img = np.ascontiguousarray(image.transpose(2, 0, 1)).astype(np.float32)
    xp = np.pad(img, ((0, 0), (PAD, PAD), (PAD, PAD)))
    win = np.arange(OH)[:, None] + np.arange(K)[None, :]
    p = xp[:, win[:, None, :, None], win[None, :, None, :]]
    return p.transpose(1, 2, 0, 3, 4).reshape(L, D)


def _to6(x, rows, tiles, width):
    # (rows, D) -> (tiles, 128, DSTEP, width) lhsT/rhs layout
    return np.ascontiguousarray(
        x.T.reshape(DSTEP, 128, tiles, width).transpose(2, 1, 0, 3))


def _prepare_in_maps(image, mem_keys, mem_values, mode=None):
    q = _im2col(image)
    qpad = np.zeros((LP, D), dtype=np.float32)
    qpad[:L] = q
    qh = qpad.astype(ml_dtypes.bfloat16)
    ql = (qpad - qh.astype(np.float32)).astype(ml_dtypes.bfloat16)
    qh6 = _to6(qh, LP, MT, 128)
    ql6 = _to6(ql, LP, MT, 128)

    eyes = np.zeros((OH, K, 84), dtype=ml_dtypes.bfloat16)
    oh_i = np.arange(OH)
    for ky in range(K):
        eyes[oh_i, ky, oh_i + ky] = 1.0

    in_maps = []
    for c in range(NCORES):
        kc = mem_keys[c * NK:(c + 1) * NK]
        kcp = np.zeros((NKP, D), dtype=np.float32)
        kcp[:NK] = kc
        kh = kcp.astype(ml_dtypes.bfloat16)
        kl = (kcp - kh.astype(np.float32)).astype(ml_dtypes.bfloat16)

        bb = np.empty(NB, dtype=ml_dtypes.bfloat16)
        vc = mem_values[c * NK:(c + 1) * NK].astype(ml_dtypes.bfloat16)
        bb[OV:OV + NK * D] = vc.reshape(-1)
        bb[OV + NK * D:OV + SZ_VALS] = 0          # zero row
        bb[OE:OE + SZ_EYES] = eyes.reshape(-1)
        bb[OQH:OQH + SZ_Q] = qh6.reshape(-1)
        bb[OQL:OQL + SZ_Q] = ql6.reshape(-1)
        bb[OKH:OKH + SZ_K] = _to6(kh, NKP, NSUB, SUBW).reshape(-1)
        bb[OKL:OKL + SZ_K] = _to6(kl, NKP, NSUB, SUBW).reshape(-1)

        fb = np.empty(NF, dtype=np.float32)
        n2 = (kc.astype(np.float64) ** 2).sum(axis=1)
        bias = np.full(NKP, -1.0e9, dtype=np.float32)
        bias[:NK] = (-0.5 * n2).astype(np.float32)
        fb[OB:OB + 128 * NKP] = np.broadcast_to(bias, (128, NKP)).reshape(-1)
        bases = (c * NK + np.arange(NSUB, dtype=np.float32) * SUBW)
        fb[OBA:OBA + 128 * NSUB] = np.broadcast_to(
            bases, (128, NSUB)).reshape(-1)
        fb[OO:OO + 64] = 1.0
        bb[OF:NB] = fb.view(ml_dtypes.bfloat16)

        in_maps.append({"bb": bb})
    return in_maps


def kernel(**inputs):
    global LAST_EXEC_NS, LAST_RESULTS
    image = np.asarray(inputs["image"], dtype=np.float32)
    mem_keys = np.asarray(inputs["mem_keys"], dtype=np.float32)
    mem_values = np.asarray(inputs["mem_values"], dtype=np.float32)

    nc = _get_nc(debug_gi=DEBUG_GI)
    in_maps = _prepare_in_maps(image, mem_keys, mem_values)

    r = run_bass_kernel_spmd(nc, in_maps, list(range(NCORES)), trace=TRACE)
    LAST_EXEC_NS = r.exec_time_ns
    LAST_RESULTS = r.results
    out = r.results[0]["out"]
    return np.ascontiguousarray(out.transpose(0, 2, 1)).astype(np.float32)


if __name__ == "__main__":
    rng = np.random.default_rng(0)
    ins = dict(
        image=rng.random((H, W, C), dtype=np.float32),
        mem_keys=rng.standard_normal((N_MEM, D), dtype=np.float32),
        mem_values=rng.standard_normal((N_MEM, D), dtype=np.float32),
    )
    o = kernel(**ins)
    print("out", o.shape, o.dtype, float(o.max()), float(o.min()))
